# revision 2
# baseline (speedup 1.0000x reference)
"""Single-head causal attention (B=8, T=2048, C=1024, H=64) on 8 TRN2 NeuronCores.

Strategy: pure data parallelism — batch element b runs on core b. Each core
computes, for its [T, C] slices q_b / k_b:

    Q = q_b @ Wq ; K = k_b @ Wk ; V = k_b @ Wv          (projections)
    S = Q @ K^T / sqrt(C), causal-masked ; P = exp(S)    (no max-subtract:
    out = (P @ V) / (P @ 1)                               S is well-scaled)

Device-side layout choices (all matmuls bf16, accumulation fp32 in PSUM):
  * Host pre-transposes q/k to [C, T] so the contraction dim (C) lands on
    SBUF partitions with zero on-chip transposes for the projections.
  * Projections produce head-major Q^T/K^T [H, T]; K and V are projected in
    one pass with a fused [Wk | Wv] stationary operand.
  * Scores are computed transposed (S^T[j, i] tiles, key index j on
    partitions) so that P^T is directly the moving operand of the P @ V
    matmul — no P transposes. Softmax denominators come free by appending a
    ones column to V (row H of the output accumulator is then P @ 1).
  * exp runs on the scalar engine reading S^T straight from PSUM, with the
    1/sqrt(C) folded into the activation's free scale.
  * The [H+1, T] accumulator is PE-transposed back to [T, H+1] in 128-col
    blocks, normalized by the reciprocal of the l column, and DMA'd out.
"""

import numpy as np
import ml_dtypes

B, T, C, H = 8, 2048, 1024, 64
P = 128                # SBUF partitions
CCH = C // P           # 8 contraction chunks
NJ = T // P            # 16 key tiles
SCALE = float(C) ** -0.5

_cached = {}


def _build():
    import concourse.bass as bass
    import concourse.mybir as mybir
    import concourse.tile as tile
    from concourse import bacc

    dt = mybir.dt
    nc = bacc.Bacc("TRN2", target_bir_lowering=False, debug=False, num_devices=B)

    qT = nc.dram_tensor("qT", [C, T], dt.bfloat16, kind="ExternalInput").ap()
    kT = nc.dram_tensor("kT", [C, T], dt.bfloat16, kind="ExternalInput").ap()
    wq = nc.dram_tensor("wq", [C, H], dt.bfloat16, kind="ExternalInput").ap()
    wkv = nc.dram_tensor("wkv", [C, 2 * H], dt.bfloat16, kind="ExternalInput").ap()
    dmask = nc.dram_tensor("dmask", [P, P], dt.bfloat16, kind="ExternalInput").ap()
    idb = nc.dram_tensor("idb", [P, P], dt.bfloat16, kind="ExternalInput").ap()
    idf = nc.dram_tensor("idf", [P, P], dt.float32, kind="ExternalInput").ap()
    out = nc.dram_tensor("out", [T, H], dt.float32, kind="ExternalOutput").ap()

    EXP = mybir.ActivationFunctionType.Exp

    with tile.TileContext(nc) as tc:
        with (
            tc.tile_pool(name="consts", bufs=1) as consts,
            tc.tile_pool(name="inbuf", bufs=1) as inbuf,
            tc.tile_pool(name="proj", bufs=1) as proj,
        ):
            # ---- constant / input loads -------------------------------------
            mask_s = consts.tile([P, P], dt.bfloat16)
            idb_s = consts.tile([P, P], dt.bfloat16)
            idf_s = consts.tile([P, P], dt.float32)
            wq_s = consts.tile([P, CCH, H], dt.bfloat16)
            wkv_s = consts.tile([P, CCH, 2 * H], dt.bfloat16)
            nc.sync.dma_start(out=mask_s[:], in_=dmask[:])
            nc.sync.dma_start(out=idb_s[:], in_=idb[:])
            nc.sync.dma_start(out=idf_s[:], in_=idf[:])
            nc.sync.dma_start(out=wq_s[:], in_=wq.rearrange("(c p) h -> p c h", p=P))
            nc.sync.dma_start(out=wkv_s[:], in_=wkv.rearrange("(c p) h -> p c h", p=P))

            kT_s = inbuf.tile([P, CCH, T], dt.bfloat16)
            qT_s = inbuf.tile([P, CCH, T], dt.bfloat16)
            for c in range(CCH):
                nc.sync.dma_start(out=kT_s[:, c, :], in_=kT[P * c:P * (c + 1), :])
            for c in range(CCH):
                nc.sync.dma_start(out=qT_s[:, c, :], in_=qT[P * c:P * (c + 1), :])

            # ---- projections -------------------------------------------------
            KVT_s = proj.tile([P, T], dt.bfloat16)   # rows 0:64 K^T, 64:128 V^T
            QT_s = proj.tile([H, T], dt.bfloat16)
            V1_s = proj.tile([P, NJ, 66], dt.bfloat16)  # V natural + ones col

            with tc.tile_pool(name="ppsum", bufs=1, space="PSUM") as ppsum, \
                 tc.tile_pool(name="vtpsum", bufs=2, space="PSUM") as vtpsum:
                KVTp = ppsum.tile([P, T], dt.float32, tag="proj")
                for c in range(CCH):
                    for nb in range(4):
                        sl = slice(512 * nb, 512 * (nb + 1))
                        nc.tensor.matmul(
                            KVTp[:, sl], lhsT=wkv_s[:, c, :], rhs=kT_s[:, c, sl],
                            start=(c == 0), stop=(c == CCH - 1))
                nc.scalar.copy(out=KVT_s[:], in_=KVTp[:])

                # V natural tiles via PE transpose of the [K^T; V^T] blocks
                nc.vector.memset(V1_s[:, :, 64:66], 1.0)
                for j in range(NJ):
                    vtp = vtpsum.tile([P, P], dt.bfloat16, tag="vt")
                    nc.tensor.transpose(vtp[:], KVT_s[:, P * j:P * (j + 1)], idb_s[:])
                    nc.vector.tensor_copy(out=V1_s[:, j, 0:64], in_=vtp[:, 64:128])

                QTp = ppsum.tile([H, T], dt.float32, tag="proj")
                for c in range(CCH):
                    for nb in range(4):
                        sl = slice(512 * nb, 512 * (nb + 1))
                        nc.tensor.matmul(
                            QTp[:, sl], lhsT=wq_s[:, c, :], rhs=qT_s[:, c, sl],
                            start=(c == 0), stop=(c == CCH - 1))
                nc.scalar.copy(out=QT_s[:], in_=QTp[:])

            # ---- attention: j (key tile) outer, exact causal ----------------
            with tc.tile_pool(name="opsum", bufs=1, space="PSUM") as opsum:
                OUTp = opsum.tile([H + 1, T], dt.float32)
                with tc.tile_pool(name="spsum", bufs=2, space="PSUM") as spsum, \
                     tc.tile_pool(name="pbuf", bufs=3) as pbuf:
                    for j in range(NJ):
                        lo = P * j
                        Pt = pbuf.tile([P, T], dt.bfloat16, tag="p")
                        # process the valid range [lo, T) in 1024-col chunks
                        s = lo
                        while s < T:
                            e = min(T, (s // 1024 + 1) * 1024)
                            Sp = spsum.tile([P, 1024], dt.float32, tag="s")
                            # matmul psum output is fp32: <=512 cols per MM
                            ss = s
                            while ss < e:
                                ee = min(e, (ss // 512 + 1) * 512)
                                nc.tensor.matmul(
                                    Sp[:, ss - s:ee - s],
                                    lhsT=KVT_s[0:H, lo:lo + P],
                                    rhs=QT_s[:, ss:ee], start=True, stop=True)
                                ss = ee
                            nc.scalar.activation(
                                out=Pt[:, s:e], in_=Sp[:, 0:e - s],
                                func=EXP, scale=SCALE)
                            if s == lo:
                                # zero strictly-upper part of diagonal block
                                nc.vector.tensor_mul(
                                    Pt[:, lo:lo + P], Pt[:, lo:lo + P], mask_s[:])
                            for w in range(s // 512, (e - 1) // 512 + 1):
                                ps = max(512 * w, s)
                                pe = 512 * (w + 1)
                                nc.tensor.matmul(
                                    OUTp[:, ps:pe], lhsT=V1_s[:, j, 0:65],
                                    rhs=Pt[:, ps:pe],
                                    start=(j == 0), stop=(j == 4 * w + 3))
                            s = e

                # ---- epilogue: transpose back, normalize, store -------------
                with tc.tile_pool(name="ebuf", bufs=1) as ebuf, \
                     tc.tile_pool(name="tpsum", bufs=2, space="PSUM") as tpsum, \
                     tc.tile_pool(name="obuf", bufs=3) as obuf:
                    outsb = ebuf.tile([H + 1, T], dt.float32)
                    nc.scalar.copy(out=outsb[:], in_=OUTp[:])
                    for t in range(NJ):
                        tp = tpsum.tile([P, H + 1], dt.float32, tag="t")
                        nc.tensor.transpose(
                            tp[:], outsb[:, P * t:P * (t + 1)],
                            idf_s[0:H + 1, 0:H + 1])
                        linv = obuf.tile([P, 1], dt.float32, tag="l")
                        nc.vector.reciprocal(linv[:], tp[:, H:H + 1])
                        ot = obuf.tile([P, H], dt.float32, tag="o")
                        nc.vector.tensor_scalar_mul(ot[:], tp[:, 0:H], linv[:])
                        nc.sync.dma_start(out=out[P * t:P * (t + 1), :], in_=ot[:])

    nc.compile()
    return nc


def _get_nc():
    if "nc" not in _cached:
        _cached["nc"] = _build()
    return _cached["nc"]


def kernel(q, k, Wq, Wk, Wv):
    from concourse.bass_utils import run_bass_kernel_spmd

    nc = _get_nc()
    bf16 = ml_dtypes.bfloat16

    wq_h = np.ascontiguousarray(Wq.astype(bf16))
    wkv_h = np.ascontiguousarray(np.concatenate([Wk, Wv], axis=1).astype(bf16))
    dmask_h = np.triu(np.ones((P, P), dtype=np.float32)).astype(bf16)
    id_h = np.eye(P, dtype=np.float32)
    idb_h = id_h.astype(bf16)

    in_maps = []
    for b in range(B):
        in_maps.append({
            "qT": np.ascontiguousarray(q[b].T.astype(bf16)),
            "kT": np.ascontiguousarray(k[b].T.astype(bf16)),
            "wq": wq_h,
            "wkv": wkv_h,
            "dmask": dmask_h,
            "idb": idb_h,
            "idf": id_h,
        })
    res = run_bass_kernel_spmd(nc, in_maps, list(range(B)))
    return np.stack([res.results[b]["out"] for b in range(B)]).astype(np.float32)


if __name__ == "__main__":
    rng = np.random.default_rng(0)
    q = rng.standard_normal((B, T, C)).astype(np.float32)
    k = rng.standard_normal((B, T, C)).astype(np.float32)
    Wq = (rng.standard_normal((C, H)) * 0.02).astype(np.float32)
    Wk = (rng.standard_normal((C, H)) * 0.02).astype(np.float32)
    Wv = (rng.standard_normal((C, H)) * 0.02).astype(np.float32)
    o = kernel(q, k, Wq, Wk, Wv)
    print("out", o.shape, o.dtype, float(np.abs(o).max()))


# revision 7
# speedup vs baseline: 1.2394x; 1.2394x over previous
"""Single-head causal attention (B=8, T=2048, C=1024, H=64) on 8 TRN2 NeuronCores.

Strategy: pure data parallelism — batch element b runs on core b. Each core
computes, for its [T, C] slices q_b / k_b:

    Q = q_b @ Wq ; K = k_b @ Wk ; V = k_b @ Wv          (projections)
    S = Q @ K^T / sqrt(C), causal-masked ; P = exp(S)    (no max-subtract:
    out = (P @ V) / (P @ 1)                               S is well-scaled)

Device-side layout (all matmuls bf16, fp32 PSUM accumulation):
  * Host pre-transposes q/k to [C, T] and pre-blocks them [tb, p, c, t] so
    each 512-column block arrives in one fully-contiguous DMA with
    8KB-per-partition lines, and the contraction dim (C) lands on SBUF
    partitions with zero on-chip input transposes.
  * Projections produce head-major Q^T/K^T [H, T]; K and V are projected in
    one pass with a fused [Wk | Wv] stationary operand; V natural tiles come
    from 16 PE transposes of the [K^T; V^T] blocks.
  * Scores are computed transposed (S^T[j, i] tiles, key index j on
    partitions) so P^T is directly the moving operand of the P @ V matmul —
    no P transposes. Softmax denominators come free via a ones column
    appended to V (row H of the accumulator is P @ 1).
  * exp runs on the scalar engine straight out of PSUM with 1/sqrt(C) folded
    into the activation's free scale.
  * The whole kernel is a single software pipeline over 512-column i-blocks:
    DMA block -> project block -> score/exp/accumulate -> transpose back,
    normalize, store. This keeps TensorE dense (HAM stays at K=8/8) and
    overlaps the input DMA with compute.
"""

import numpy as np
import ml_dtypes

B, T, C, H = 8, 2048, 1024, 64
P = 128                  # SBUF partitions
CCH = C // P             # 8 contraction chunks
NJ = T // P              # 16 key tiles of 128
NB = T // 512            # 4 column blocks of 512
SCALE = float(C) ** -0.5

_cached = {}


def _build():
    import concourse.bass as bass
    import concourse.mybir as mybir
    import concourse.tile as tile
    from concourse import bacc

    dt = mybir.dt
    nc = bacc.Bacc("TRN2", target_bir_lowering=False, debug=False, num_devices=B)

    # blocked inputs: [tb, p, c, t] so one DMA per 512-col block is contiguous
    qT = nc.dram_tensor("qT", [NB, P, CCH, 512], dt.bfloat16, kind="ExternalInput").ap()
    kT = nc.dram_tensor("kT", [NB, P, CCH, 512], dt.bfloat16, kind="ExternalInput").ap()
    wq = nc.dram_tensor("wq", [C, H], dt.bfloat16, kind="ExternalInput").ap()
    wkv = nc.dram_tensor("wkv", [C, 2 * H], dt.bfloat16, kind="ExternalInput").ap()
    dmask = nc.dram_tensor("dmask", [P, P], dt.bfloat16, kind="ExternalInput").ap()
    idb = nc.dram_tensor("idb", [P, P], dt.bfloat16, kind="ExternalInput").ap()
    idf = nc.dram_tensor("idf", [P, P], dt.float32, kind="ExternalInput").ap()
    out = nc.dram_tensor("out", [T, H], dt.float32, kind="ExternalOutput").ap()

    EXP = mybir.ActivationFunctionType.Exp

    with tile.TileContext(nc) as tc:
        with (
            tc.tile_pool(name="consts", bufs=1) as consts,
            tc.tile_pool(name="inbuf", bufs=1) as inbuf,
            tc.tile_pool(name="proj", bufs=1) as proj,
            tc.tile_pool(name="ppsum", bufs=1, space="PSUM") as ppsum,
            tc.tile_pool(name="vtpsum", bufs=1, space="PSUM") as vtpsum,
            tc.tile_pool(name="opsum", bufs=2, space="PSUM") as opsum,
            tc.tile_pool(name="spsum", bufs=3, space="PSUM") as spsum,
            tc.tile_pool(name="pbuf", bufs=6) as pbuf,
            tc.tile_pool(name="ebuf", bufs=2) as ebuf,
            tc.tile_pool(name="obuf", bufs=4) as obuf,
        ):
            # ---- constants ---------------------------------------------------
            mask_s = consts.tile([P, P], dt.bfloat16)
            idb_s = consts.tile([P, P], dt.bfloat16)
            idf_s = consts.tile([P, P], dt.float32)
            wq_s = consts.tile([P, CCH, H], dt.bfloat16)
            wkv_s = consts.tile([P, CCH, 2 * H], dt.bfloat16)
            nc.sync.dma_start(out=mask_s[:], in_=dmask[:])
            nc.sync.dma_start(out=idb_s[:], in_=idb[:])
            nc.sync.dma_start(out=idf_s[:], in_=idf[:])
            nc.sync.dma_start(out=wq_s[:], in_=wq.rearrange("(c p) h -> p c h", p=P))
            nc.sync.dma_start(out=wkv_s[:], in_=wkv.rearrange("(c p) h -> p c h", p=P))

            kT_s = inbuf.tile([P, NB, CCH, 512], dt.bfloat16)
            qT_s = inbuf.tile([P, NB, CCH, 512], dt.bfloat16)
            KVT_s = proj.tile([P, T], dt.bfloat16)   # rows 0:64 K^T, 64:128 V^T
            QT_s = proj.tile([H, T], dt.bfloat16)
            V1_s = proj.tile([P, NJ, 66], dt.bfloat16)  # V natural + ones col
            nc.vector.memset(V1_s[:, :, 64:66], 1.0)

            # ---- per-block input DMA + projections (pipelined) --------------
            for tb in range(NB):
                sl = slice(512 * tb, 512 * (tb + 1))
                nc.sync.dma_start(out=kT_s[:, tb], in_=kT[tb])
                nc.sync.dma_start(out=qT_s[:, tb], in_=qT[tb])

                KVTp = ppsum.tile([P, 512], dt.float32, tag="kvt")
                for c in range(CCH):
                    nc.tensor.matmul(KVTp[:], lhsT=wkv_s[:, c, :],
                                     rhs=kT_s[:, tb, c, :],
                                     start=(c == 0), stop=(c == CCH - 1))
                nc.vector.tensor_copy(out=KVT_s[:, sl], in_=KVTp[:])

                for jj in range(4):
                    j = 4 * tb + jj
                    vtp = vtpsum.tile([P, P], dt.bfloat16, tag="vt")
                    nc.tensor.transpose(
                        vtp[:], KVT_s[:, P * j:P * (j + 1)], idb_s[:])
                    nc.vector.tensor_copy(out=V1_s[:, j, 0:64], in_=vtp[:, 64:128])

                QTp = ppsum.tile([H, 512], dt.float32, tag="qt")
                for c in range(CCH):
                    nc.tensor.matmul(QTp[:], lhsT=wq_s[:, c, :],
                                     rhs=qT_s[:, tb, c, :],
                                     start=(c == 0), stop=(c == CCH - 1))
                nc.vector.tensor_copy(out=QT_s[:, sl], in_=QTp[:])

            # ---- attention + epilogue, pipelined over i-blocks --------------
            for ic in range(NB):
                ilo = 512 * ic
                OUTp = opsum.tile([H + 1, 512], dt.float32, tag="out")
                for j in range(4 * ic + 4):
                    lo = max(P * j, ilo)       # global start col of this chunk
                    n = 512 * (ic + 1) - lo    # chunk width (512 or less @diag)
                    Sp = spsum.tile([P, 512], dt.float32, tag="s")
                    nc.tensor.matmul(Sp[:, 0:n], lhsT=KVT_s[0:H, P * j:P * (j + 1)],
                                     rhs=QT_s[:, lo:lo + n], start=True, stop=True)
                    Pt = pbuf.tile([P, 512], dt.bfloat16, tag="p")
                    nc.scalar.activation(out=Pt[:, 0:n], in_=Sp[:, 0:n],
                                         func=EXP, scale=SCALE)
                    if j >= 4 * ic:
                        # diagonal block: zero strictly-upper 128x128 triangle
                        nc.vector.tensor_mul(Pt[:, 0:P], Pt[:, 0:P], mask_s[:])
                    nc.tensor.matmul(OUTp[:, lo - ilo:512], lhsT=V1_s[:, j, 0:65],
                                     rhs=Pt[:, 0:n],
                                     start=(j == 0), stop=(j == 4 * ic + 3))

                # transpose back, normalize by the l row, store
                outsb = ebuf.tile([H + 1, 512], dt.float32, tag="e")
                nc.vector.tensor_copy(out=outsb[:], in_=OUTp[:])
                for t in range(4):
                    gt = 4 * ic + t
                    tp = vtpsum.tile([P, H + 1], dt.float32, tag="vt")
                    nc.tensor.transpose(
                        tp[:], outsb[:, P * t:P * (t + 1)],
                        idf_s[0:H + 1, 0:H + 1])
                    linv = obuf.tile([P, 1], dt.float32, tag="l")
                    nc.vector.reciprocal(linv[:], tp[:, H:H + 1])
                    ot = obuf.tile([P, H], dt.float32, tag="o")
                    nc.vector.tensor_scalar_mul(ot[:], tp[:, 0:H], linv[:])
                    nc.sync.dma_start(out=out[P * gt:P * (gt + 1), :], in_=ot[:])

    nc.compile()
    return nc


def _get_nc():
    if "nc" not in _cached:
        _cached["nc"] = _build()
    return _cached["nc"]


def _block(xT):
    """[C, T] -> [NB, P, CCH, 512] so each 512-col block is contiguous."""
    return np.ascontiguousarray(
        xT.reshape(CCH, P, NB, 512).transpose(2, 1, 0, 3))


def _host_inputs(q, k, Wq, Wk, Wv):
    bf16 = ml_dtypes.bfloat16
    wq_h = np.ascontiguousarray(Wq.astype(bf16))
    wkv_h = np.ascontiguousarray(np.concatenate([Wk, Wv], axis=1).astype(bf16))
    dmask_h = np.triu(np.ones((P, P), dtype=np.float32)).astype(bf16)
    id_h = np.eye(P, dtype=np.float32)
    in_maps = []
    for b in range(B):
        in_maps.append({
            "qT": _block(q[b].T.astype(bf16)),
            "kT": _block(k[b].T.astype(bf16)),
            "wq": wq_h,
            "wkv": wkv_h,
            "dmask": dmask_h,
            "idb": id_h.astype(bf16),
            "idf": id_h,
        })
    return in_maps


def kernel(q, k, Wq, Wk, Wv):
    from concourse.bass_utils import run_bass_kernel_spmd

    nc = _get_nc()
    in_maps = _host_inputs(q, k, Wq, Wk, Wv)
    res = run_bass_kernel_spmd(nc, in_maps, list(range(B)))
    return np.stack([res.results[b]["out"] for b in range(B)]).astype(np.float32)


if __name__ == "__main__":
    rng = np.random.default_rng(0)
    q = rng.standard_normal((B, T, C)).astype(np.float32)
    k = rng.standard_normal((B, T, C)).astype(np.float32)
    Wq = (rng.standard_normal((C, H)) * 0.02).astype(np.float32)
    Wk = (rng.standard_normal((C, H)) * 0.02).astype(np.float32)
    Wv = (rng.standard_normal((C, H)) * 0.02).astype(np.float32)
    o = kernel(q, k, Wq, Wk, Wv)
    print("out", o.shape, o.dtype, float(np.abs(o).max()))


# revision 9
# speedup vs baseline: 1.3148x; 1.0609x over previous
"""Single-head causal attention (B=8, T=2048, C=1024, H=64) on 8 TRN2 NeuronCores.

Strategy: pure data parallelism — batch element b runs on core b. Each core
computes, for its [T, C] slices q_b / k_b:

    Q = q_b @ Wq ; K = k_b @ Wk ; V = k_b @ Wv          (projections)
    S = Q @ K^T / sqrt(C), causal-masked ; P = exp(S)    (no max-subtract:
    out = (P @ V) / (P @ 1)                               S is well-scaled)

Device-side layout (all matmuls bf16, fp32 PSUM accumulation):
  * Host pre-transposes q/k to [C, T] and pre-blocks them [tb, p, c, t] so
    each 512-column block arrives in one fully-contiguous DMA with
    8KB-per-partition lines, and the contraction dim (C) lands on SBUF
    partitions with zero on-chip input transposes.
  * Projections produce head-major Q^T/K^T [H, T]; K and V are projected in
    one pass with a fused [Wk | Wv] stationary operand; V natural tiles come
    from 16 PE transposes of the [K^T; V^T] blocks.
  * Scores are computed transposed (S^T[j, i] tiles, key index j on
    partitions) so P^T is directly the moving operand of the P @ V matmul —
    no P transposes. Softmax denominators come free via a ones column
    appended to V (row H of the accumulator is P @ 1).
  * exp runs on the scalar engine straight out of PSUM with 1/sqrt(C) folded
    into the activation's free scale.
  * The whole kernel is a single software pipeline over 512-column i-blocks:
    DMA block -> project block -> score/exp/accumulate -> transpose back,
    normalize, store. This keeps TensorE dense (HAM stays at K=8/8) and
    overlaps the input DMA with compute.
"""

import numpy as np
import ml_dtypes

B, T, C, H = 8, 2048, 1024, 64
P = 128                  # SBUF partitions
CCH = C // P             # 8 contraction chunks
NJ = T // P              # 16 key tiles of 128
NB = T // 512            # 4 column blocks of 512
SCALE = float(C) ** -0.5

_cached = {}


def _build():
    import concourse.bass as bass
    import concourse.mybir as mybir
    import concourse.tile as tile
    from concourse import bacc

    dt = mybir.dt
    nc = bacc.Bacc("TRN2", target_bir_lowering=False, debug=False, num_devices=B)

    # blocked inputs: [tb, p, c, t] so one DMA per 512-col block is contiguous
    qT = nc.dram_tensor("qT", [NB, P, CCH, 512], dt.bfloat16, kind="ExternalInput").ap()
    kT = nc.dram_tensor("kT", [NB, P, CCH, 512], dt.bfloat16, kind="ExternalInput").ap()
    wq = nc.dram_tensor("wq", [C, H], dt.bfloat16, kind="ExternalInput").ap()
    wkv = nc.dram_tensor("wkv", [C, 2 * H], dt.bfloat16, kind="ExternalInput").ap()
    dmask = nc.dram_tensor("dmask", [P, P], dt.bfloat16, kind="ExternalInput").ap()
    idb = nc.dram_tensor("idb", [P, P], dt.bfloat16, kind="ExternalInput").ap()
    idf = nc.dram_tensor("idf", [P, P], dt.float32, kind="ExternalInput").ap()
    out = nc.dram_tensor("out", [T, H], dt.float32, kind="ExternalOutput").ap()

    EXP = mybir.ActivationFunctionType.Exp

    with tile.TileContext(nc) as tc:
        with (
            tc.tile_pool(name="consts", bufs=1) as consts,
            tc.tile_pool(name="inbuf", bufs=1) as inbuf,
            tc.tile_pool(name="proj", bufs=1) as proj,
            tc.tile_pool(name="ppsum", bufs=1, space="PSUM") as ppsum,
            tc.tile_pool(name="vtpsum", bufs=1, space="PSUM") as vtpsum,
            tc.tile_pool(name="opsum", bufs=2, space="PSUM") as opsum,
            tc.tile_pool(name="spsum", bufs=3, space="PSUM") as spsum,
            tc.tile_pool(name="pbuf", bufs=6) as pbuf,
            tc.tile_pool(name="ebuf", bufs=2) as ebuf,
            tc.tile_pool(name="obuf", bufs=4) as obuf,
        ):
            # ---- constants ---------------------------------------------------
            mask_s = consts.tile([P, P], dt.bfloat16)
            idb_s = consts.tile([P, P], dt.bfloat16)
            idf_s = consts.tile([P, P], dt.float32)
            wq_s = consts.tile([P, CCH, H], dt.bfloat16)
            wkv_s = consts.tile([P, CCH, 2 * H], dt.bfloat16)
            # constants go on the gpsimd (SWDGE) ring so they don't head-block
            # the big input transfers on the sync (HWDGE) ring
            nc.gpsimd.dma_start(out=mask_s[:], in_=dmask[:])
            nc.gpsimd.dma_start(out=idb_s[:], in_=idb[:])
            nc.gpsimd.dma_start(out=idf_s[:], in_=idf[:])
            nc.gpsimd.dma_start(out=wq_s[:], in_=wq.rearrange("(c p) h -> p c h", p=P))
            nc.gpsimd.dma_start(out=wkv_s[:], in_=wkv.rearrange("(c p) h -> p c h", p=P))

            kT_s = inbuf.tile([P, NB, CCH, 512], dt.bfloat16)
            qT_s = inbuf.tile([P, NB, CCH, 512], dt.bfloat16)
            KVT_s = proj.tile([P, T], dt.bfloat16)   # rows 0:64 K^T, 64:128 V^T
            QT_s = proj.tile([H, T], dt.bfloat16)
            V1_s = proj.tile([P, NJ, 66], dt.bfloat16)  # V natural + ones col
            nc.vector.memset(V1_s[:, :, 64:66], 1.0)

            # ---- pipeline stages --------------------------------------------
            def proj_block(tb):
                """DMA one 512-col block of k/q and project it."""
                sl = slice(512 * tb, 512 * (tb + 1))
                nc.sync.dma_start(out=kT_s[:, tb], in_=kT[tb])
                nc.sync.dma_start(out=qT_s[:, tb], in_=qT[tb])

                KVTp = ppsum.tile([P, 512], dt.float32, tag="kvt")
                for c in range(CCH):
                    nc.tensor.matmul(KVTp[:], lhsT=wkv_s[:, c, :],
                                     rhs=kT_s[:, tb, c, :],
                                     start=(c == 0), stop=(c == CCH - 1))
                nc.vector.tensor_copy(out=KVT_s[:, sl], in_=KVTp[:])

                for jj in range(4):
                    j = 4 * tb + jj
                    vtp = vtpsum.tile([P, P], dt.bfloat16, tag="vt")
                    nc.tensor.transpose(
                        vtp[:], KVT_s[:, P * j:P * (j + 1)], idb_s[:])
                    nc.vector.tensor_copy(out=V1_s[:, j, 0:64], in_=vtp[:, 64:128])

                QTp = ppsum.tile([H, 512], dt.float32, tag="qt")
                for c in range(CCH):
                    nc.tensor.matmul(QTp[:], lhsT=wq_s[:, c, :],
                                     rhs=qT_s[:, tb, c, :],
                                     start=(c == 0), stop=(c == CCH - 1))
                nc.vector.tensor_copy(out=QT_s[:, sl], in_=QTp[:])

            def attn_block(ic):
                """Score/exp/accumulate + epilogue for one 512-col i-block."""
                ilo = 512 * ic
                OUTp = opsum.tile([H + 1, 512], dt.float32, tag="out")
                for j in range(4 * ic + 4):
                    lo = max(P * j, ilo)       # global start col of this chunk
                    n = 512 * (ic + 1) - lo    # chunk width (512 or less @diag)
                    Sp = spsum.tile([P, 512], dt.float32, tag="s")
                    nc.tensor.matmul(Sp[:, 0:n], lhsT=KVT_s[0:H, P * j:P * (j + 1)],
                                     rhs=QT_s[:, lo:lo + n], start=True, stop=True)
                    Pt = pbuf.tile([P, 512], dt.bfloat16, tag="p")
                    nc.scalar.activation(out=Pt[:, 0:n], in_=Sp[:, 0:n],
                                         func=EXP, scale=SCALE)
                    if j >= 4 * ic:
                        # diagonal block: zero strictly-upper 128x128 triangle
                        nc.vector.tensor_mul(Pt[:, 0:P], Pt[:, 0:P], mask_s[:])
                    nc.tensor.matmul(OUTp[:, lo - ilo:512], lhsT=V1_s[:, j, 0:65],
                                     rhs=Pt[:, 0:n],
                                     start=(j == 0), stop=(j == 4 * ic + 3))

                # transpose back, normalize by the l row, store
                outsb = ebuf.tile([H + 1, 512], dt.float32, tag="e")
                nc.vector.tensor_copy(out=outsb[:], in_=OUTp[:])
                for t in range(4):
                    gt = 4 * ic + t
                    tp = vtpsum.tile([P, H + 1], dt.float32, tag="vt")
                    nc.tensor.transpose(
                        tp[:], outsb[:, P * t:P * (t + 1)],
                        idf_s[0:H + 1, 0:H + 1])
                    linv = obuf.tile([P, 1], dt.float32, tag="l")
                    nc.vector.reciprocal(linv[:], tp[:, H:H + 1])
                    ot = obuf.tile([P, H], dt.float32, tag="o")
                    nc.vector.tensor_scalar_mul(ot[:], tp[:, 0:H], linv[:])
                    nc.sync.dma_start(out=out[P * gt:P * (gt + 1), :], in_=ot[:])

            # Interleave projection blocks with attention blocks so TensorE
            # stays dense during the exp-paced attention stretches (keeps the
            # HAM clock gate at 8/8) and DMA overlaps compute.
            proj_block(0)
            proj_block(1)
            attn_block(0)
            proj_block(2)
            attn_block(1)
            proj_block(3)
            attn_block(2)
            attn_block(3)

    nc.compile()
    return nc


def _get_nc():
    if "nc" not in _cached:
        _cached["nc"] = _build()
    return _cached["nc"]


def _block(xT):
    """[C, T] -> [NB, P, CCH, 512] so each 512-col block is contiguous."""
    return np.ascontiguousarray(
        xT.reshape(CCH, P, NB, 512).transpose(2, 1, 0, 3))


def _host_inputs(q, k, Wq, Wk, Wv):
    bf16 = ml_dtypes.bfloat16
    wq_h = np.ascontiguousarray(Wq.astype(bf16))
    wkv_h = np.ascontiguousarray(np.concatenate([Wk, Wv], axis=1).astype(bf16))
    dmask_h = np.triu(np.ones((P, P), dtype=np.float32)).astype(bf16)
    id_h = np.eye(P, dtype=np.float32)
    in_maps = []
    for b in range(B):
        in_maps.append({
            "qT": _block(q[b].T.astype(bf16)),
            "kT": _block(k[b].T.astype(bf16)),
            "wq": wq_h,
            "wkv": wkv_h,
            "dmask": dmask_h,
            "idb": id_h.astype(bf16),
            "idf": id_h,
        })
    return in_maps


def kernel(q, k, Wq, Wk, Wv):
    from concourse.bass_utils import run_bass_kernel_spmd

    nc = _get_nc()
    in_maps = _host_inputs(q, k, Wq, Wk, Wv)
    res = run_bass_kernel_spmd(nc, in_maps, list(range(B)))
    return np.stack([res.results[b]["out"] for b in range(B)]).astype(np.float32)


if __name__ == "__main__":
    rng = np.random.default_rng(0)
    q = rng.standard_normal((B, T, C)).astype(np.float32)
    k = rng.standard_normal((B, T, C)).astype(np.float32)
    Wq = (rng.standard_normal((C, H)) * 0.02).astype(np.float32)
    Wk = (rng.standard_normal((C, H)) * 0.02).astype(np.float32)
    Wv = (rng.standard_normal((C, H)) * 0.02).astype(np.float32)
    o = kernel(q, k, Wq, Wk, Wv)
    print("out", o.shape, o.dtype, float(np.abs(o).max()))
